# revision 1
# baseline (speedup 1.0000x reference)
"""Trainium2 Bass kernel for the FFF (fast feedforward / MoE-routing) module.

Math (per token x of dim 1024, PAR=8 trees of 255 nodes):
  logits = x @ W_in.T + b_in                      # [B, 2040]
  dec    = logits > 0
  acts   = silu(logits)
  dmap   = indicator of the 8 visited nodes per tree (root + 7 descents,
           descending by dec at the current node)
  out    = (acts * dmap) @ W_out.T                # [B, 1024]

Strategy (8 NeuronCores, data-parallel over the 8192 tokens, 1024 each):
  - GEMM1 in bf16 hi/lo split: 3 passes (hi*hi + hi*lo + lo*hi) for the
    decision-relevant node levels 0..6 (cols 0..1020), 1 pass (hi*hi) for the
    leaf level (cols 1020..2040) whose sign is never used.  PSUM accumulates
    fp32; the fp32 bias is added on the vector engine, so decision signs are
    ~fp32-accurate.
  - dmap is built level-by-level with strided vector ops in a node-major
    column layout (col = 8*node + tree): child1 = V_d * dec_d (stride-2
    upsample), child0 = V_d - child1.
  - masked acts cast to fp16, transposed 128x128 on the PE, GEMM2 in fp16
    (exact products, fp32 PSUM accumulation).
  - weight DMAs are chunked and emitted in need-order so the PE starts
    within a few us instead of waiting for the full 13.6 MB weight load.
"""

import numpy as np
import ml_dtypes

DIM = 1024
PAR = 8
DEPTH = 7
N_NODES = 255
WIDTH = PAR * N_NODES          # 2040
NODES_PAD = 2048               # pad masked-acts/W_out^T to 16*128
N_CORES = 8
TOK_PER_CORE = 1024
TT = 128                       # tokens per tile
NTILES = TOK_PER_CORE // TT    # 8
NT_W = 510                     # GEMM1 n-tile width (4 * 510 = 2040)
K_CH = DIM // 128              # 8 contraction chunks for GEMM1
C_CH = NODES_PAD // 128        # 16 contraction chunks for GEMM2
DEC_COLS = 8 * 127             # 1016: decision nodes are levels 0..6

_PROGRAM = None


def _build_program():
    import concourse.bacc as bacc
    import concourse.tile as tile
    from concourse import mybir
    from concourse.masks import make_identity
    import concourse.bass as bass

    f32 = mybir.dt.float32
    bf16 = mybir.dt.bfloat16
    f16 = mybir.dt.float16
    Alu = mybir.AluOpType
    Act = mybir.ActivationFunctionType

    nc = bacc.Bacc("TRN2", target_bir_lowering=False, debug=False,
                   num_devices=N_CORES)

    # Per-core DRAM I/O (layouts chosen so every DMA has long contiguous
    # runs); xt packs the bf16 hi/lo split as [...,0/1,...]
    xt = nc.dram_tensor("xt", [128, NTILES, 2, K_CH, TT], bf16,
                        kind="ExternalInput")
    w1_hi = nc.dram_tensor("w1_hi", [128, K_CH, WIDTH], bf16,
                           kind="ExternalInput")
    # lo-part only needed for the decision region (cols 0..1020)
    w1_lo = nc.dram_tensor("w1_lo", [128, K_CH, 2 * NT_W], bf16,
                           kind="ExternalInput")
    b1 = nc.dram_tensor("b1", [WIDTH], f32, kind="ExternalInput")
    w2 = nc.dram_tensor("w2", [128, C_CH, DIM], f16, kind="ExternalInput")
    y = nc.dram_tensor("y", [TOK_PER_CORE, DIM], f32, kind="ExternalOutput")

    with tile.TileContext(nc) as tc:
        with (
            tc.tile_pool(name="wts", bufs=1) as wts,
            tc.tile_pool(name="xts", bufs=3) as xts,
            tc.tile_pool(name="logits", bufs=2) as logits_pool,
            tc.tile_pool(name="mask", bufs=2) as mask_pool,
            tc.tile_pool(name="acts", bufs=2) as acts_pool,
            tc.tile_pool(name="out", bufs=2) as out_pool,
            tc.tile_pool(name="pl", bufs=4, space="PSUM") as pl_pool,
            tc.tile_pool(name="pt", bufs=2, space="PSUM") as pt_pool,
            tc.tile_pool(name="py", bufs=2, space="PSUM") as py_pool,
        ):
            # ---- resident weights (DMAs emitted in need-order below) ----
            w1h_sb = wts.tile([128, K_CH, WIDTH], bf16)
            w1l_sb = wts.tile([128, K_CH, 2 * NT_W], bf16)
            w2_sb = wts.tile([128, C_CH, DIM], f16)
            b1_sb = wts.tile([128, WIDTH], f32)
            ident = wts.tile([128, 128], f16)

            xt_tiles = {}

            def prefetch_xt(j, eng=None):
                xhl = xts.tile([128, 2, K_CH, TT], bf16, tag="x")
                (eng or nc.sync).dma_start(out=xhl, in_=xt[:, j, :, :, :])
                xt_tiles[j] = xhl

            # Weight DMAs chunked and emitted on the Sync engine in
            # consumption order (only Sync's HW DGE fans out over all 16
            # queues, ~400 GB/s; each dma_start dispatch costs ~0.6us).
            # x-tile prefetches ride GpSimd's slower SW DGE — their
            # deadlines are generous and this keeps Sync free for weights.
            nc.sync.dma_start(out=w1h_sb[:, 0, :], in_=w1_hi[:, 0, :])
            xhl0 = xts.tile([128, 2, K_CH, TT], bf16, tag="x")
            nc.sync.dma_start(out=xhl0[:, 0], in_=xt[:, 0, 0, :, :])
            nc.sync.dma_start(out=xhl0[:, 1], in_=xt[:, 0, 1, :, :])
            xt_tiles[0] = xhl0
            nc.sync.dma_start(out=w1l_sb[:, 0, :], in_=w1_lo[:, 0, :])
            nc.sync.dma_start(out=w1h_sb[:, 1, :], in_=w1_hi[:, 1, :])
            nc.sync.dma_start(out=w1l_sb[:, 1, :], in_=w1_lo[:, 1, :])
            for k in range(2, K_CH, 2):
                nc.sync.dma_start(out=w1h_sb[:, k:k + 2, :],
                                  in_=w1_hi[:, k:k + 2, :])
                nc.sync.dma_start(out=w1l_sb[:, k:k + 2, :],
                                  in_=w1_lo[:, k:k + 2, :])
            prefetch_xt(1)
            for c in range(0, C_CH, 4):
                nc.sync.dma_start(out=w2_sb[:, c:c + 4, :],
                                  in_=w2[:, c:c + 4, :])
            # bias broadcast rides GpSimd's SW DGE, off the weight path
            b1_bcast = bass.AP(tensor=b1, offset=0, ap=[[0, 128], [1, WIDTH]])
            nc.gpsimd.dma_start(out=b1_sb, in_=b1_bcast)
            make_identity(nc, ident)
            for c in range(C_CH):
                nc.sync.dma_start(out=w2_sb[:, c, :], in_=w2[:, c, :])

            # per-token-tile masked-acts, produced by stage A (GEMM1+mask),
            # consumed by stage B (transpose + GEMM2); 1-deep software
            # pipeline so the PE never waits on the vector-engine epilogue.
            state = {}

            def stage_a(j):
                if j not in xt_tiles:
                    prefetch_xt(j, nc.gpsimd)
                xhl = xt_tiles.pop(j)
                xh, xl = xhl[:, 0], xhl[:, 1]

                lg = logits_pool.tile([TT, WIDTH], f32, tag="lg")
                d1 = mask_pool.tile([TT, DEC_COLS], f16, tag="d1")
                vv = mask_pool.tile([TT, WIDTH], f16, tag="vv")
                ac = acts_pool.tile([TT, WIDTH], f16, tag="ac")
                mk = acts_pool.tile([TT, NODES_PAD], f16, tag="mk")

                for nt in range(4):
                    nsl = slice(nt * NT_W, (nt + 1) * NT_W)
                    pl = pl_pool.tile([TT, NT_W], f32)
                    npass = 3 if nt < 2 else 1
                    nmm = K_CH * npass
                    i = 0
                    for k in range(K_CH):
                        mms = [(xh, w1h_sb)]
                        if npass == 3:
                            mms += [(xh, w1l_sb), (xl, w1h_sb)]
                        for (xx, ww) in mms:
                            nc.tensor.matmul(
                                pl, lhsT=xx[:, k, :], rhs=ww[:, k, nsl],
                                start=(i == 0), stop=(i == nmm - 1))
                            i += 1
                    # bias add (fp32, exact) PSUM -> SBUF
                    nc.vector.tensor_tensor(lg[:, nsl], pl, b1_sb[:, nsl],
                                            Alu.add)
                    if nt == 0:
                        nc.vector.tensor_scalar(
                            d1[:, 0:NT_W], lg[:, 0:NT_W], 0.0, None,
                            Alu.is_gt)
                    elif nt == 1:
                        nc.vector.tensor_scalar(
                            d1[:, NT_W:DEC_COLS], lg[:, NT_W:DEC_COLS],
                            0.0, None, Alu.is_gt)
                    nc.scalar.activation(ac[:, nsl], lg[:, nsl], Act.Silu)

                # tree mask: V_0 = 1 at root cols; then per level
                # child1 = V_d * dec_d, child0 = V_d - child1
                nc.vector.memset(vv[:, 0:8], 1.0)
                for d in range(DEPTH):
                    ld = 8 * (1 << d)
                    c0 = 8 * ((1 << d) - 1)
                    c1 = 8 * ((1 << (d + 1)) - 1)
                    vpar = vv[:, c0:c0 + ld].rearrange("p (i t) -> p i t", t=8)
                    dpar = d1[:, c0:c0 + ld].rearrange("p (i t) -> p i t", t=8)
                    kids = vv[:, c1:c1 + 2 * ld].rearrange(
                        "p (i two t) -> p i two t", two=2, t=8)
                    nc.vector.tensor_tensor(kids[:, :, 1, :], vpar, dpar,
                                            Alu.mult)
                    nc.vector.tensor_tensor(kids[:, :, 0, :], vpar,
                                            kids[:, :, 1, :], Alu.subtract)

                # masked acts (fp16); cols 2040:2048 are zero padding so the
                # last transpose/GEMM2 chunk is a uniform 128 wide
                nc.vector.memset(mk[:, WIDTH:NODES_PAD], 0.0)
                nc.vector.tensor_tensor(mk[:, 0:1024], ac[:, 0:1024],
                                        vv[:, 0:1024], Alu.mult)
                nc.vector.tensor_tensor(mk[:, 1024:WIDTH], ac[:, 1024:WIDTH],
                                        vv[:, 1024:WIDTH], Alu.mult)
                state[j] = mk

            def stage_b(j):
                mk = state.pop(j)
                at = acts_pool.tile([128, C_CH, TT], f16, tag="at")
                # transpose in groups -> one PSUM tile -> one copy; first
                # group is a single chunk so GEMM2 can start immediately
                c = 0
                for gsz in (1, 3, 4, 4, 4):
                    pt = pt_pool.tile([128, 512], f16)
                    for i in range(gsz):
                        nc.tensor.transpose(
                            pt[:, i * 128:(i + 1) * 128],
                            mk[:, (c + i) * 128:(c + i + 1) * 128], ident)
                    nc.scalar.copy(
                        at[:, c:c + gsz, :],
                        pt[:, :gsz * 128].rearrange("p (c t) -> p c t", t=TT))
                    c += gsz
                ys = out_pool.tile([TT, DIM], f32, tag="ys")
                for h in range(2):
                    hs = slice(h * 512, (h + 1) * 512)
                    py = py_pool.tile([TT, 512], f32)
                    for c in range(C_CH):
                        nc.tensor.matmul(
                            py, lhsT=at[:, c, :], rhs=w2_sb[:, c, hs],
                            start=(c == 0), stop=(c == C_CH - 1))
                    nc.vector.tensor_copy(ys[:, hs], py)
                    nc.sync.dma_start(out=y[j * TT:(j + 1) * TT, hs],
                                      in_=ys[:, hs])

            # software pipeline: A(0), A(1), B(0), A(2), B(1), ... B(7)
            stage_a(0)
            for j in range(1, NTILES):
                stage_a(j)
                stage_b(j - 1)
            stage_b(NTILES - 1)

    nc.finalize()
    return nc


def _get_program():
    global _PROGRAM
    if _PROGRAM is None:
        _PROGRAM = _build_program()
    return _PROGRAM


def _split_hi_lo(a):
    hi = a.astype(ml_dtypes.bfloat16)
    lo = (a - hi.astype(np.float32)).astype(ml_dtypes.bfloat16)
    return hi, lo


def kernel(oldx, W_in, b_in, W_out):
    from concourse.bass_utils import run_bass_kernel_spmd

    oldx = np.asarray(oldx)
    W_in = np.asarray(W_in, dtype=np.float32)
    b_in = np.asarray(b_in, dtype=np.float32)
    W_out = np.asarray(W_out, dtype=np.float32)
    x = oldx.reshape(-1, DIM).astype(np.float32)          # [8192, 1024]

    # node-major column permutation: our col 8n+t  <-  ref col 255t+n
    i = np.arange(WIDTH)
    perm = 255 * (i % PAR) + (i // PAR)

    w1t = W_in[perm, :].T.astype(np.float32)              # [1024, 2040]
    w1t_hi, w1t_lo = _split_hi_lo(w1t)
    # [dim, width] -> [128, K_CH, WIDTH] with dim = k*128 + p
    w1_hi = np.ascontiguousarray(
        w1t_hi.reshape(K_CH, 128, WIDTH).transpose(1, 0, 2))
    w1_lo = np.ascontiguousarray(
        w1t_lo.reshape(K_CH, 128, WIDTH).transpose(1, 0, 2)[:, :, :2 * NT_W])
    b1 = np.ascontiguousarray(b_in[perm])

    w2t = np.zeros((NODES_PAD, DIM), np.float32)
    w2t[:WIDTH] = W_out.T[perm, :]
    w2 = np.ascontiguousarray(
        w2t.astype(np.float16).reshape(C_CH, 128, DIM).transpose(1, 0, 2))

    in_maps = []
    for c in range(N_CORES):
        xc = x[c * TOK_PER_CORE:(c + 1) * TOK_PER_CORE]   # [1024, 1024]
        xt_hi, xt_lo = _split_hi_lo(xc.T)                 # [dim, tok]
        # [dim, tok] -> [128, NTILES, K_CH, TT]; dim = k*128+p, tok = j*128+t
        xt_hi = xt_hi.reshape(K_CH, 128, NTILES, TT).transpose(1, 2, 0, 3)
        xt_lo = xt_lo.reshape(K_CH, 128, NTILES, TT).transpose(1, 2, 0, 3)
        xt = np.ascontiguousarray(np.stack([xt_hi, xt_lo], axis=2))
        in_maps.append({
            "xt": xt, "w1_hi": w1_hi, "w1_lo": w1_lo,
            "b1": b1, "w2": w2,
        })

    nc = _get_program()
    res = run_bass_kernel_spmd(nc, in_maps, core_ids=list(range(N_CORES)))
    out = np.concatenate([res.results[c]["y"] for c in range(N_CORES)],
                         axis=0)
    return out.reshape(oldx.shape).astype(np.float32)



# revision 2
# speedup vs baseline: 1.2416x; 1.2416x over previous
"""Trainium2 Bass kernel for the FFF (fast feedforward / MoE-routing) module.

Math (per token x of dim 1024, PAR=8 trees of 255 nodes):
  logits = x @ W_in.T + b_in                      # [B, 2040]
  dec    = logits > 0
  acts   = silu(logits)
  dmap   = indicator of the 8 visited nodes per tree (root + 7 descents,
           descending by dec at the current node)
  out    = (acts * dmap) @ W_out.T                # [B, 1024]

Strategy (8 NeuronCores, data-parallel over the 8192 tokens, 1024 each):
  - GEMM1 in fp16 with a precision ladder keyed to how much a decision
    flip at each tree level costs (a flip at level d replaces the 7-d
    deeper visited nodes, i.e. token rel-err ~ sqrt(2(7-d)/64)):
      levels 0-4 (cols 0:248):    3-pass  xh@wh + xh@wl + xl@wh
      levels 5-6 (cols 248:1016): 1-pass  xh@wh   (fp16, sigma~2.3e-4)
      leaves     (cols 1016:2040): 1-pass xh@wh   (values only)
    with xh=f16(x), xl=f16(x-xh), wh=f16(w), wl=bf16(w-wh).  fp32 bias
    added on the vector engine.  Empirical global rel-err ~1.1e-2.
  - dmap is built level-by-level with strided vector ops in a node-major
    column layout (col = 8*node + tree): child1 = V_d * dec_d, child0 =
    V_d - child1.
  - masked acts (fp16) are transposed by the DMA XBAR (dma_start_transpose,
    one instruction per token tile, ~1.8us, zero PE cost); GEMM2 runs in
    fp16 off the transposed layout.  W_out rows are pre-permuted on the
    host to match the XBAR's [p, c] fold of the 2048 node columns.
  - weight DMAs are packed into few large-descriptor dma_starts and
    dual-issued on the two HWDGE queues (Sync + Activation) in need-order
    so the PE starts within ~10us.
"""

import numpy as np
import ml_dtypes

DIM = 1024
PAR = 8
DEPTH = 7
N_NODES = 255
WIDTH = PAR * N_NODES          # 2040
NODES_PAD = 2048               # pad masked-acts/W_out^T to 16*128
N_CORES = 8
TOK_PER_CORE = 1024
TT = 128                       # tokens per tile
NTILES = TOK_PER_CORE // TT    # 8
K_CH = DIM // 128              # 8 contraction chunks for GEMM1
C_CH = NODES_PAD // 128        # 16 contraction chunks for GEMM2
DEC_COLS = 8 * 127             # 1016: decision nodes are levels 0..6
NA = 248                       # 3-pass region: levels 0..4 (cols 0:248)
NB = WIDTH - NA                # 1792: 1-pass region
# GEMM1 n-tiles: (start, width, npass)
NT_SPEC = [(0, 248, 3), (248, 512, 1), (760, 512, 1), (1272, 512, 1),
           (1784, 256, 1)]

_PROGRAM = None


def _build_program():
    import concourse.bacc as bacc
    import concourse.tile as tile
    from concourse import mybir
    import concourse.bass as bass

    f32 = mybir.dt.float32
    bf16 = mybir.dt.bfloat16
    f16 = mybir.dt.float16
    Alu = mybir.AluOpType
    Act = mybir.ActivationFunctionType

    nc = bacc.Bacc("TRN2", target_bir_lowering=False, debug=False,
                   num_devices=N_CORES)

    # Per-core DRAM I/O; xt packs the fp16 hi/lo split as [...,0/1,...]
    xt = nc.dram_tensor("xt", [128, NTILES, 2, K_CH, TT], f16,
                        kind="ExternalInput")
    w1ha = nc.dram_tensor("w1ha", [128, K_CH, NA], f16, kind="ExternalInput")
    w1la = nc.dram_tensor("w1la", [128, K_CH, NA], bf16, kind="ExternalInput")
    w1hb = nc.dram_tensor("w1hb", [128, K_CH, NB], f16, kind="ExternalInput")
    b1 = nc.dram_tensor("b1", [WIDTH], f32, kind="ExternalInput")
    w2 = nc.dram_tensor("w2", [128, C_CH, DIM], f16, kind="ExternalInput")
    y = nc.dram_tensor("y", [TOK_PER_CORE, DIM], f32, kind="ExternalOutput")

    with tile.TileContext(nc) as tc:
        with (
            tc.tile_pool(name="wts", bufs=1) as wts,
            tc.tile_pool(name="xts", bufs=3) as xts,
            tc.tile_pool(name="logits", bufs=2) as logits_pool,
            tc.tile_pool(name="mask", bufs=2) as mask_pool,
            tc.tile_pool(name="acts", bufs=2) as acts_pool,
            tc.tile_pool(name="out", bufs=2) as out_pool,
            tc.tile_pool(name="pl", bufs=5, space="PSUM") as pl_pool,
            tc.tile_pool(name="py", bufs=2, space="PSUM") as py_pool,
        ):
            # ---- resident weights (DMAs emitted in need-order below) ----
            w1ha_sb = wts.tile([128, K_CH, NA], f16)
            w1la_sb = wts.tile([128, K_CH, NA], bf16)
            w1hb_sb = wts.tile([128, K_CH, NB], f16)
            w2_sb = wts.tile([128, C_CH, DIM], f16)
            b1_sb = wts.tile([128, WIDTH], f32)

            xt_tiles = {}

            def prefetch_xt(j, eng=None):
                xhl = xts.tile([128, 2, K_CH, TT], f16, tag="x")
                (eng or nc.gpsimd).dma_start(out=xhl, in_=xt[:, j, :, :, :])
                xt_tiles[j] = xhl

            # Need-order weight loads, dual-issued on the two HWDGE engines
            # (Sync + Activation).  First matmul needs xt0 + slab A.
            xhl0 = xts.tile([128, 2, K_CH, TT], f16, tag="x")
            nc.sync.dma_start(out=xhl0, in_=xt[:, 0, :, :, :])
            xt_tiles[0] = xhl0
            nc.scalar.dma_start(out=w1ha_sb, in_=w1ha[:, :, :])
            nc.scalar.dma_start(out=w1la_sb, in_=w1la[:, :, :])
            # 1-pass region, in n-tile consumption order (2 chunks)
            nc.sync.dma_start(out=w1hb_sb[:, :, 0:768],
                              in_=w1hb[:, :, 0:768])
            nc.sync.dma_start(out=w1hb_sb[:, :, 768:NB],
                              in_=w1hb[:, :, 768:NB])
            # bias broadcast (needed at first bias add)
            b1_bcast = bass.AP(tensor=b1, offset=0, ap=[[0, 128], [1, WIDTH]])
            nc.scalar.dma_start(out=b1_sb, in_=b1_bcast)
            prefetch_xt(1, nc.scalar)
            # w2 in 4 chunks so the first XBAR transpose doesn't queue
            # behind 1024 descriptors
            for c in range(0, C_CH, 4):
                nc.sync.dma_start(out=w2_sb[:, c:c + 4, :],
                                  in_=w2[:, c:c + 4, :])
            prefetch_xt(2, nc.scalar)

            # per-token-tile transposed masked-acts, produced by stage A
            # (GEMM1+mask+XBAR), consumed by stage B (GEMM2); 1-deep
            # software pipeline so the PE never waits on the epilogue.
            state = {}

            def stage_a(j):
                if j not in xt_tiles:
                    prefetch_xt(j)
                xhl = xt_tiles.pop(j)
                xh, xl = xhl[:, 0], xhl[:, 1]

                lg = logits_pool.tile([TT, WIDTH], f32, tag="lg")
                d1 = mask_pool.tile([TT, DEC_COLS], f16, tag="d1")
                vv = mask_pool.tile([TT, WIDTH], f16, tag="vv")
                ac = acts_pool.tile([TT, WIDTH], f16, tag="ac")
                mk = acts_pool.tile([TT, NODES_PAD], f16, tag="mk")
                at = acts_pool.tile([128, C_CH, TT], f16, tag="at")

                for (n0, nw, npass) in NT_SPEC:
                    nsl = slice(n0, n0 + nw)
                    pl = pl_pool.tile([TT, nw], f32)
                    nmm = K_CH * npass
                    i = 0
                    for k in range(K_CH):
                        if npass == 3:
                            mms = [(xh, w1ha_sb[:, k, :]),
                                   (xh, w1la_sb[:, k, :]),
                                   (xl, w1ha_sb[:, k, :])]
                        else:
                            mms = [(xh, w1hb_sb[:, k, n0 - NA:n0 - NA + nw])]
                        for (xx, ww) in mms:
                            nc.tensor.matmul(pl, lhsT=xx[:, k, :], rhs=ww,
                                             start=(i == 0),
                                             stop=(i == nmm - 1))
                            i += 1
                    # bias add (fp32, exact) PSUM -> SBUF
                    nc.vector.tensor_tensor(lg[:, nsl], pl, b1_sb[:, nsl],
                                            Alu.add)
                    # decision bits for cols < DEC_COLS
                    if n0 < DEC_COLS:
                        de = min(n0 + nw, DEC_COLS)
                        nc.vector.tensor_scalar(
                            d1[:, n0:de], lg[:, n0:de], 0.0, None, Alu.is_gt)
                    nc.scalar.activation(ac[:, nsl], lg[:, nsl], Act.Silu)

                # tree mask: V_0 = 1 at root cols; then per level
                # child1 = V_d * dec_d, child0 = V_d - child1
                nc.vector.memset(vv[:, 0:8], 1.0)
                for d in range(DEPTH):
                    ld = 8 * (1 << d)
                    c0 = 8 * ((1 << d) - 1)
                    c1 = 8 * ((1 << (d + 1)) - 1)
                    vpar = vv[:, c0:c0 + ld].rearrange("p (i t) -> p i t", t=8)
                    dpar = d1[:, c0:c0 + ld].rearrange("p (i t) -> p i t", t=8)
                    kids = vv[:, c1:c1 + 2 * ld].rearrange(
                        "p (i two t) -> p i two t", two=2, t=8)
                    nc.vector.tensor_tensor(kids[:, :, 1, :], vpar, dpar,
                                            Alu.mult)
                    nc.vector.tensor_tensor(kids[:, :, 0, :], vpar,
                                            kids[:, :, 1, :], Alu.subtract)

                # masked acts (fp16); cols 2040:2048 are zero padding so the
                # XBAR transpose input is a uniform [128, 2048]
                nc.vector.memset(mk[:, WIDTH:NODES_PAD], 0.0)
                nc.vector.tensor_tensor(mk[:, 0:1024], ac[:, 0:1024],
                                        vv[:, 0:1024], Alu.mult)
                nc.vector.tensor_tensor(mk[:, 1024:WIDTH], ac[:, 1024:WIDTH],
                                        vv[:, 1024:WIDTH], Alu.mult)
                # XBAR transpose: at[p, c, t] = mk[t, sigma(p, c)]; w2 rows
                # are host-permuted by the same sigma.
                nc.scalar.dma_start_transpose(out=at, in_=mk[:, :])
                state[j] = at

            def stage_b(j, last=False):
                at = state.pop(j)
                ys = out_pool.tile([TT, DIM], f32, tag="ys")
                for h in range(2):
                    hs = slice(h * 512, (h + 1) * 512)
                    py = py_pool.tile([TT, 512], f32)
                    for c in range(C_CH):
                        nc.tensor.matmul(
                            py, lhsT=at[:, c, :], rhs=w2_sb[:, c, hs],
                            start=(c == 0), stop=(c == C_CH - 1))
                    nc.vector.tensor_copy(ys[:, hs], py)
                    if last:
                        nc.sync.dma_start(out=y[j * TT:(j + 1) * TT, hs],
                                          in_=ys[:, hs])
                if not last:
                    nc.sync.dma_start(out=y[j * TT:(j + 1) * TT, :],
                                      in_=ys)

            # software pipeline: A(0), A(1), B(0), A(2), B(1), ... B(7)
            stage_a(0)
            for j in range(1, NTILES):
                stage_a(j)
                stage_b(j - 1)
            stage_b(NTILES - 1, last=True)

    nc.finalize()
    return nc


def _get_program():
    global _PROGRAM
    if _PROGRAM is None:
        _PROGRAM = _build_program()
    return _PROGRAM


def _split_hi_lo_f16(a):
    hi = a.astype(np.float16)
    lo = (a - hi.astype(np.float32)).astype(np.float16)
    return hi, lo


def kernel(oldx, W_in, b_in, W_out):
    from concourse.bass_utils import run_bass_kernel_spmd

    oldx = np.asarray(oldx)
    W_in = np.asarray(W_in, dtype=np.float32)
    b_in = np.asarray(b_in, dtype=np.float32)
    W_out = np.asarray(W_out, dtype=np.float32)
    x = oldx.reshape(-1, DIM).astype(np.float32)          # [8192, 1024]

    # node-major column permutation: our col 8n+t  <-  ref col 255t+n
    i = np.arange(WIDTH)
    perm = 255 * (i % PAR) + (i // PAR)

    w1t = W_in[perm, :].T.astype(np.float32)              # [1024, 2040]
    w1t_hi = w1t.astype(np.float16).astype(np.float32)
    w1t_lo = (w1t - w1t_hi).astype(ml_dtypes.bfloat16)
    # [dim, width] -> [128, K_CH, cols] with dim = k*128 + p
    w1h = w1t_hi.astype(np.float16).reshape(K_CH, 128, WIDTH)
    w1l = w1t_lo.reshape(K_CH, 128, WIDTH)
    w1ha = np.ascontiguousarray(w1h[:, :, :NA].transpose(1, 0, 2))
    w1la = np.ascontiguousarray(w1l[:, :, :NA].transpose(1, 0, 2))
    w1hb = np.ascontiguousarray(w1h[:, :, NA:].transpose(1, 0, 2))
    b1 = np.ascontiguousarray(b_in[perm])

    w2t = np.zeros((NODES_PAD, DIM), np.float32)
    w2t[:WIDTH] = W_out.T[perm, :]
    # XBAR fold (probed): at[p, c, t] = mk[t, 128*c + p] -> natural chunk
    # transpose; w2[p, c, :] = W2T_pad[128*c + p, :]
    w2 = np.ascontiguousarray(
        w2t.astype(np.float16).reshape(C_CH, 128, DIM).transpose(1, 0, 2))

    in_maps = []
    for c in range(N_CORES):
        xc = x[c * TOK_PER_CORE:(c + 1) * TOK_PER_CORE]   # [1024, 1024]
        xt_hi, xt_lo = _split_hi_lo_f16(xc.T)             # [dim, tok]
        # [dim, tok] -> [128, NTILES, K_CH, TT]; dim = k*128+p, tok = j*128+t
        xt_hi = xt_hi.reshape(K_CH, 128, NTILES, TT).transpose(1, 2, 0, 3)
        xt_lo = xt_lo.reshape(K_CH, 128, NTILES, TT).transpose(1, 2, 0, 3)
        xtc = np.ascontiguousarray(np.stack([xt_hi, xt_lo], axis=2))
        in_maps.append({
            "xt": xtc, "w1ha": w1ha, "w1la": w1la, "w1hb": w1hb,
            "b1": b1, "w2": w2,
        })

    nc = _get_program()
    res = run_bass_kernel_spmd(nc, in_maps, core_ids=list(range(N_CORES)))
    out = np.concatenate([res.results[c]["y"] for c in range(N_CORES)],
                         axis=0)
    return out.reshape(oldx.shape).astype(np.float32)
